# revision 1
# baseline (speedup 1.0000x reference)
"""Trainium2 Bass kernel v3 for nn_EntropyBasedLossBase (joint-KDE-histogram entropies).

Sharding: data parallel over batch B=8 across 8 NeuronCores (one sample-row
pair per core, N=131072 reshaped to [128 partitions, 1024]).

v3 = v1's PROVEN matmul geometry (512 pair-matmuls, contiguous single-free-dim
lhsT/rhs slices, RW=130 rhs tiles with two trailing ones columns accumulating
R1) + the v2 expansion cost cuts re-expressed in chunk-major layout:
- staircase t-tiles built per BIN-BLOCK: one broadcast-z TensorTensor for the
  lowest b0 bins, then (NB/b0 - 1) packed f16 TensorScalar "+b0*blk" adds
  (4x DVE mode) - the expensive broadcast op covers b0/64 of the tile.
- single fused clamp TensorScalar (max,min - 4x mode) per signal tile.
- a tunable subset of units runs the clamp as a ScalarE Relu-pair instead.
- the z' compact pass runs in f16 (magic 1536 round) with zp16 = z'+1
  replacing the second iota.
- NO gpsimd tensor ops (10us/op on HW) and NO multi-free-dim matmul APs
  (rejected/slow on HW): those were the two v2 sim-vs-HW traps.
"""
import sys

sys.path.insert(0, "/opt/trn_rl_repo")

from contextlib import ExitStack

import numpy as np

import concourse.bacc as bacc
import concourse.bass as bass
import concourse.bass_isa as bass_isa
import concourse.tile as tile
from concourse import mybir
from concourse.bass_utils import run_bass_kernel_spmd

F32 = mybir.dt.float32
F16 = mybir.dt.float16
OP = mybir.AluOpType
ACT = mybir.ActivationFunctionType

NB = 64            # num bins
P = 128            # partitions
NCOL = 1024        # free dim of the compact [128, 1024] layout (N = P*NCOL)
EPS = float(np.finfo(np.float32).eps)
MAGIC16 = 1536.0   # 1.5 * 2^10: float16 round-to-int shift constant


def _spread_seq(cnt, total=32):
    if cnt <= 0:
        return set()
    step = total / cnt
    s = set()
    x = 0.0
    while len(s) < cnt:
        s.add(min(total - 1, int(x)))
        x += step
    return s


def build_nc(repeat=1, gch=64, npsum=6, act_clamps=6, eb=3, sb=4, b0=16,
             do_mm=True):
    GCH = gch
    NGROUP = NCOL // GCH
    NPAIR = GCH // 2
    NU = 2 * NGROUP
    W = NB * GCH                  # dense t-tile width (chunk-major: c*NB + r)
    RW = 2 * NB + 2               # rhs width per pair (128 S cols + 2 ones)
    nc = bacc.Bacc("TRN2", num_devices=8)

    sig1 = nc.dram_tensor("sig1", [P, NCOL], F32, kind="ExternalInput")
    sig2 = nc.dram_tensor("sig2", [P, NCOL], F32, kind="ExternalInput")
    c_dt = nc.dram_tensor("c_dt", [NB, NB], F32, kind="ExternalInput")
    out_h = nc.dram_tensor("out_h", [1, 4], F32, kind="ExternalOutput")

    act_set = _spread_seq(act_clamps, NU)

    with ExitStack() as ctx:
        tc = ctx.enter_context(tile.TileContext(nc))
        singles = ctx.enter_context(tc.tile_pool(name="singles", bufs=1))
        comp = ctx.enter_context(tc.tile_pool(name="comp", bufs=1))
        texp = ctx.enter_context(tc.tile_pool(name="texp", bufs=eb))
        apool = ctx.enter_context(tc.tile_pool(name="apool", bufs=2))
        sexp = ctx.enter_context(tc.tile_pool(name="sexp", bufs=eb))
        psum = ctx.enter_context(tc.tile_pool(name="psum", bufs=1, space="PSUM"))
        post = ctx.enter_context(tc.tile_pool(name="post", bufs=1))
        postp = ctx.enter_context(tc.tile_pool(name="postp", bufs=1, space="PSUM"))

        # ---- constants ----
        # chunk-major iota: value (r + 1) at offset c*NB + r
        iota1 = singles.tile([P, W], F16)
        nc.gpsimd.iota(iota1[:], pattern=[[0, GCH], [1, NB]], base=1,
                       channel_multiplier=0, allow_small_or_imprecise_dtypes=True)
        iota0d = singles.tile([P, 16 * GCH], F16)
        nc.gpsimd.iota(iota0d[:], pattern=[[0, GCH], [1, 16]], base=1,
                       channel_multiplier=0, allow_small_or_imprecise_dtypes=True)
        s2bufs = []
        for sb_i in range(sb):
            s2b = singles.tile([P, NPAIR * RW], F16, name=f"s2buf{sb_i}")
            ones_ap = bass.AP(s2b.tensor, s2b.offset + 2 * NB,
                              [s2b.ap[0], [RW, NPAIR], [1, 2]])
            nc.vector.memset(ones_ap, 1.0)
            s2bufs.append(s2b)
        dtm = singles.tile([NB, NB], F32)
        nc.sync.dma_start(out=dtm[:], in_=c_dt.ap())
        ones_col = singles.tile([NB, 1], F32)
        nc.vector.memset(ones_col[:], 1.0)

        def z_ap(zt, k, g, nb=NB):
            """chunk-major z operand: [[1, GCH], [0, nb]] (bcast over bins)."""
            ap = zt[:, k * NCOL + g * GCH: k * NCOL + g * GCH + GCH]
            return bass.AP(ap.tensor, ap.offset, [ap.ap[0], [1, GCH], [0, nb]])

        def slab(t, r0, nb_):
            """bins [r0, r0+nb_) of all chunks in a chunk-major [P, W] tile."""
            return bass.AP(t.tensor, t.offset + r0, [t.ap[0], [NB, GCH], [1, nb_]])

        for _rep in range(repeat):
            # ---- load + per-sample compact pass (both signals) ----
            comb = comp.tile([P, 2 * NCOL], F16, tag="comb")   # s for both signals
            for k, sig in enumerate((sig1, sig2)):
                v = comp.tile([P, NCOL], F32, tag=f"v{k}")
                nc.sync.dma_start(out=v[:], in_=sig.ap())

                mx1 = comp.tile([1, 1], F32, tag=f"mx1{k}")
                mn1 = comp.tile([1, 1], F32, tag=f"mn1{k}")
                nc.gpsimd.tensor_reduce(out=mx1[:], in_=v[:], axis=mybir.AxisListType.XYZWC, op=OP.max)
                nv = comp.tile([P, NCOL], F32, tag=f"nv{k}")
                nc.scalar.activation(out=nv[:], in_=v[:], func=ACT.Copy, scale=-1.0)
                nc.gpsimd.tensor_reduce(out=mn1[:], in_=nv[:], axis=mybir.AxisListType.XYZWC, op=OP.max)
                mxa = comp.tile([P, 1], F32, tag=f"mxa{k}")
                mnn = comp.tile([P, 1], F32, tag=f"mnn{k}")
                nc.gpsimd.partition_broadcast(mxa[:], mx1[:])
                nc.gpsimd.partition_broadcast(mnn[:], mn1[:])
                mna = comp.tile([P, 1], F32, tag=f"mna{k}")
                nc.vector.tensor_scalar(out=mna[:], in0=mnn[:], scalar1=-1.0, scalar2=None, op0=OP.mult)

                diff = comp.tile([P, 1], F32, tag=f"diff{k}")
                nc.vector.tensor_tensor(out=diff[:], in0=mxa[:], in1=mna[:], op=OP.subtract)
                rdiff = comp.tile([P, 1], F32, tag=f"rdiff{k}")
                nc.vector.reciprocal(out=rdiff[:], in_=diff[:])
                guard = comp.tile([P, 1], F32, tag=f"guard{k}")
                nc.vector.tensor_scalar(out=guard[:], in0=diff[:], scalar1=EPS, scalar2=None, op0=OP.is_gt)
                rs = comp.tile([P, 1], F32, tag=f"rs{k}")
                nc.vector.tensor_scalar(out=rs[:], in0=rdiff[:], scalar1=float(NB), scalar2=None, op0=OP.mult)
                nc.vector.tensor_tensor(out=rs[:], in0=rs[:], in1=guard[:], op=OP.mult)

                # s = (v - mn) * rscale in [0, 64]  (f16 out)
                nc.vector.tensor_scalar(out=comb[:, k * NCOL:(k + 1) * NCOL], in0=v[:],
                                        scalar1=mna[:], scalar2=rs[:],
                                        op0=OP.subtract, op1=OP.mult)

            # z' = s + 0.9u - 1.8u|u|, u = frac(s) - 0.5; f16 chain, nu = -u
            b1 = comp.tile([P, 2 * NCOL], F16, tag="B")
            nc.vector.tensor_scalar(out=b1[:], in0=comb[:], scalar1=MAGIC16 - 0.5,
                                    scalar2=None, op0=OP.add)
            bb = comp.tile([P, 2 * NCOL], F16, tag="C")
            nc.vector.tensor_scalar(out=bb[:], in0=b1[:], scalar1=-MAGIC16 + 0.5,
                                    scalar2=None, op0=OP.add)   # rhe(s-.5)+.5
            nu = comp.tile([P, 2 * NCOL], F16, tag="D")
            nc.vector.tensor_tensor(out=nu[:], in0=bb[:], in1=comb[:], op=OP.subtract)  # = -u
            au = comp.tile([P, 2 * NCOL], F16, tag="E")
            nc.scalar.activation(out=au[:], in_=nu[:], func=ACT.Abs)      # = |u|
            t1c = comp.tile([P, 2 * NCOL], F16, tag="B")
            nc.vector.tensor_tensor(out=t1c[:], in0=nu[:], in1=au[:], op=OP.mult)  # = -u|u|
            v1c = comp.tile([P, 2 * NCOL], F16, tag="C")
            nc.vector.tensor_scalar(out=v1c[:], in0=nu[:], scalar1=-0.9, scalar2=None, op0=OP.mult)
            v2c = comp.tile([P, 2 * NCOL], F16, tag="E")
            nc.vector.tensor_scalar(out=v2c[:], in0=t1c[:], scalar1=1.8, scalar2=None, op0=OP.mult)
            v3c = comp.tile([P, 2 * NCOL], F16, tag="D")
            nc.vector.tensor_tensor(out=v3c[:], in0=v1c[:], in1=v2c[:], op=OP.add)
            zc16 = comp.tile([P, 2 * NCOL], F16, tag="zc16")
            nc.vector.tensor_tensor(out=zc16[:], in0=comb[:], in1=v3c[:], op=OP.add)
            zp16 = comp.tile([P, 2 * NCOL], F16, tag="zp16")
            nc.vector.tensor_scalar(out=zp16[:], in0=zc16[:], scalar1=1.0, scalar2=None,
                                    op0=OP.add)   # z' + 1

            if not do_mm:
                hout = post.tile([1, 4], F32, tag="hout_ab")
                nc.vector.memset(hout[:], 0.0)
                nc.sync.dma_start(out=out_h.ap(), in_=hout[:])
                continue

            # ---- expansion + matmul over groups ----
            mps = []
            for j in range(npsum):
                mtile = psum.tile([P, RW], F32, tag=f"mps{j}", name=f"mps{j}")
                mps.append(mtile)
            n_mm = NGROUP * NPAIR
            mm_idx = 0
            for g in range(NGROUP):
                outs = {}
                korder = sorted(range(2), key=lambda k: (2 * g + k) not in act_set)
                for k in korder:
                    u = 2 * g + k
                    on_act = u in act_set
                    if k == 0:
                        st = sexp.tile([P, W], F16, tag="s1t")
                        out_ap = st[:]
                    else:
                        st = s2bufs[g % sb]
                        out_ap = bass.AP(st.tensor, st.offset,
                                         [st.ap[0], [RW, NPAIR], [1, 2 * NB]])
                    tt = texp.tile([P, W], F16, tag=f"t{k}")
                    t0d = texp.tile([P, b0 * GCH], F16, tag=f"t0d{k}")

                    def emit_sub(form, zsrc):
                        """form 'iz': t = iota1 - z; 'zi': t = z - iota1.
                        Broadcast-z TT with DENSE out (t0d, bins < b0), then
                        packed dense->slab copy/adds for each bin-block."""
                        nblk = NB // b0
                        zap = z_ap(zsrc, k, g, b0)
                        if form == 'iz':
                            i0, i1 = iota0d[:], zap
                        else:
                            i0, i1 = zap, iota0d[:]
                        nc.vector.tensor_tensor(out=t0d[:], in0=i0, in1=i1,
                                                op=OP.subtract)
                        nc.vector.tensor_copy(out=slab(tt, 0, b0), in_=t0d[:])
                        for blk in range(1, nblk):
                            d = float(b0 * blk) * (1.0 if form == 'iz' else -1.0)
                            nc.vector.tensor_scalar(out=slab(tt, blk * b0, b0),
                                                    in0=t0d[:], scalar1=d,
                                                    scalar2=None, op0=OP.add)

                    if on_act:
                        # t = iota1 - z (k=1) or (z+1) - iota1 (k=0);
                        # ACT pair gives clamp(1 - t, 0, 1)
                        emit_sub('zi' if k == 0 else 'iz', zp16 if k == 0 else zc16)
                        at = apool.tile([P, W], F16, tag="a")
                        nc.scalar.activation(out=at[:], in_=tt[:], func=ACT.Relu)
                        nc.scalar.activation(out=out_ap, in_=at[:], func=ACT.Relu,
                                             bias=1.0, scale=-1.0)
                    else:
                        # t = iota1 - z (k=0) or (z+1) - iota1 (k=1); DVE clamp
                        emit_sub('iz' if k == 0 else 'zi', zc16 if k == 0 else zp16)
                        nc.vector.tensor_scalar(out=out_ap, in0=tt[:], scalar1=0.0,
                                                scalar2=1.0, op0=OP.max, op1=OP.min)
                    outs[k] = st
                s1t, s2t = outs[0], outs[1]
                for m in range(NPAIR):
                    j = mm_idx % npsum
                    nc.tensor.matmul(
                        out=mps[j][:],
                        lhsT=s1t[:, m * 2 * NB:(m + 1) * 2 * NB],
                        rhs=s2t[:, m * RW:(m + 1) * RW],
                        start=(mm_idx < npsum), stop=(mm_idx >= n_mm - npsum),
                    )
                    mm_idx += 1

            # ---- combine psum tiles ----
            acc = post.tile([P, RW], F32)
            nc.vector.tensor_copy(out=acc[:], in_=mps[0][:])
            for j in range(1, len(mps)):
                nc.vector.tensor_tensor(out=acc[:], in0=mps[j][:], in1=acc[:], op=OP.add)
            accb = post.tile([NB, NB + 2], F32)
            nc.sync.dma_start(out=accb[:], in_=acc[NB:P, NB:RW])
            # Mt = block(0,0) + block(1,1)
            msb = post.tile([NB, NB + 1], F32)
            nc.vector.memset(msb[:, 0:1], 0.0)
            nc.vector.tensor_tensor(out=msb[:, 1:NB + 1], in0=acc[0:NB, 0:NB],
                                    in1=accb[:, 0:NB], op=OP.add)
            # jcr = [coldiff(Mt) | R1]
            jcr = post.tile([NB, NB + 1], F32)
            nc.vector.tensor_tensor(out=jcr[:, 0:NB], in0=msb[:, 1:NB + 1], in1=msb[:, 0:NB],
                                    op=OP.subtract)
            nc.vector.tensor_tensor(out=jcr[:, NB:NB + 1], in0=acc[0:NB, 2 * NB:2 * NB + 1],
                                    in1=accb[:, NB:NB + 1], op=OP.add)
            # [D coldiff(Mt) | D R1]
            jps = postp.tile([NB, NB + 1], F32)
            nc.tensor.matmul(out=jps[:], lhsT=dtm[:], rhs=jcr[:], start=True, stop=True)
            jsb = post.tile([NB, NB], F32)
            # joint = (D R1) e0^T - D coldiff(Mt) D^T
            nc.vector.tensor_scalar(out=jsb[:], in0=jps[:, 0:NB], scalar1=-1.0, scalar2=None,
                                    op0=OP.mult)
            nc.vector.tensor_tensor(out=jsb[:, 0:1], in0=jps[:, NB:NB + 1], in1=jsb[:, 0:1],
                                    op=OP.add)

            # ---- clip, sums, entropies ----
            cj = post.tile([NB, NB], F32)
            rowsum = post.tile([NB, 1], F32)
            nc.vector.tensor_scalar(out=cj[:], in0=jsb[:], scalar1=EPS, scalar2=None,
                                    op0=OP.max, op1=OP.add, accum_out=rowsum[:])
            tot = post.tile([NB, 1], F32)
            nc.gpsimd.partition_all_reduce(tot[:], rowsum[:], channels=NB,
                                           reduce_op=bass_isa.ReduceOp.add)

            ly = post.tile([NB, 1], F32)
            nc.scalar.activation(out=ly[:], in_=rowsum[:], func=ACT.Ln)
            cly = post.tile([NB, 1], F32)
            nc.vector.tensor_tensor(out=cly[:], in0=rowsum[:], in1=ly[:], op=OP.mult)
            sy = post.tile([NB, 1], F32)
            nc.gpsimd.partition_all_reduce(sy[:], cly[:], channels=NB,
                                           reduce_op=bass_isa.ReduceOp.add)

            lj = post.tile([NB, NB], F32)
            nc.scalar.activation(out=lj[:], in_=cj[:], func=ACT.Ln)
            clj = post.tile([NB, NB], F32)
            rowsum_cl = post.tile([NB, 1], F32)
            nc.vector.tensor_tensor(out=clj[:], in0=cj[:], in1=lj[:], op=OP.mult)
            nc.vector.tensor_reduce(out=rowsum_cl[:], in_=clj[:], axis=mybir.AxisListType.X, op=OP.add)
            sxy = post.tile([NB, 1], F32)
            nc.gpsimd.partition_all_reduce(sxy[:], rowsum_cl[:], channels=NB,
                                           reduce_op=bass_isa.ReduceOp.add)

            pxp = postp.tile([1, NB], F32)
            nc.tensor.matmul(out=pxp[:], lhsT=ones_col[:], rhs=cj[:], start=True, stop=True)
            px = post.tile([1, NB], F32)
            nc.vector.tensor_copy(out=px[:], in_=pxp[:])
            lx = post.tile([1, NB], F32)
            nc.scalar.activation(out=lx[:], in_=px[:], func=ACT.Ln)
            clx = post.tile([1, NB], F32)
            sx = post.tile([1, 1], F32)
            nc.vector.tensor_tensor(out=clx[:], in0=px[:], in1=lx[:], op=OP.mult)
            nc.vector.tensor_reduce(out=sx[:], in_=clx[:], axis=mybir.AxisListType.X, op=OP.add)

            lnT = post.tile([1, 1], F32)
            nc.scalar.activation(out=lnT[:], in_=tot[0:1, 0:1], func=ACT.Ln)
            rT = post.tile([1, 1], F32)
            nc.vector.reciprocal(out=rT[:], in_=tot[0:1, 0:1])

            hout = post.tile([1, 4], F32)
            for col, sv in ((0, sx[0:1, 0:1]), (1, sy[0:1, 0:1]), (2, sxy[0:1, 0:1])):
                tmp = post.tile([1, 1], F32, tag=f"tmp{col}")
                nc.vector.tensor_tensor(out=tmp[:], in0=sv, in1=rT[:], op=OP.mult)
                nc.vector.tensor_tensor(out=hout[:, col:col + 1], in0=lnT[:], in1=tmp[:],
                                        op=OP.subtract)
            nc.vector.memset(hout[:, 3:4], 0.0)
            nc.sync.dma_start(out=out_h.ap(), in_=hout[:])

    nc.compile()
    return nc


BEST_KW = {"gch": 64, "npsum": 6, "act_clamps": 6, "eb": 3, "sb": 4, "b0": 16}

_NC_CACHE = {}


def _get_nc(repeat=1, **kw):
    key = (repeat, tuple(sorted(kw.items())))
    if key not in _NC_CACHE:
        _NC_CACHE[key] = build_nc(repeat, **kw)
    return _NC_CACHE[key]


def _dt_matrix():
    # c_dt[k, m] = D[m, k] with D = I - subdiag  (joint = D @ coldiff(M))
    d = np.zeros((NB, NB), np.float32)
    for k in range(NB):
        d[k, k] = 1.0
        if k + 1 < NB:
            d[k, k + 1] = -1.0
    return d


def kernel(reference_signal: np.ndarray, other_signal: np.ndarray):
    B, N = reference_signal.shape
    assert (B, N) == (8, 131072)
    nc = _get_nc(1, **BEST_KW)
    c_dt = _dt_matrix()
    in_maps = []
    for r in range(B):
        in_maps.append({
            "sig1": np.ascontiguousarray(reference_signal[r].reshape(P, NCOL)),
            "sig2": np.ascontiguousarray(other_signal[r].reshape(P, NCOL)),
            "c_dt": c_dt,
        })
    res = run_bass_kernel_spmd(nc, in_maps, list(range(8)))
    hx = np.empty(B, np.float32)
    hy = np.empty(B, np.float32)
    hxy = np.empty(B, np.float32)
    for r in range(B):
        o = res.results[r]["out_h"]
        hx[r], hy[r], hxy[r] = o[0, 0], o[0, 1], o[0, 2]
    return (hx, hy, hxy)


def _build_sharded(nc, in_maps):
    """Replicate bass2jax.run_bass_via_pjrt's jit construction, returning a
    callable + prepared args so executions can be repeated/timed."""
    import jax
    import numpy as _np
    from jax.sharding import Mesh, PartitionSpec
    from jax.experimental.shard_map import shard_map
    from concourse import bass2jax as b2j

    b2j.install_neuronx_cc_hook()
    nc_ = nc
    partition_name = nc_.partition_id_tensor.name if nc_.partition_id_tensor else None
    in_names, out_names, out_avals, zero_outs = [], [], [], []
    for alloc in nc_.m.functions[0].allocations:
        if not isinstance(alloc, mybir.MemoryLocationSet):
            continue
        name = alloc.memorylocations[0].name
        if alloc.kind == "ExternalInput":
            if name != partition_name:
                in_names.append(name)
        elif alloc.kind == "ExternalOutput":
            out_names.append(name)
            shape = tuple(alloc.tensor_shape)
            dtype = mybir.dt.np(alloc.dtype)
            out_avals.append(jax.core.ShapedArray(shape, dtype))
            zero_outs.append(_np.zeros(shape, dtype))
    n_params = len(in_names)
    n_outs = len(out_avals)
    all_in_names = list(in_names) + list(out_names)
    if partition_name is not None:
        all_in_names.append(partition_name)

    def _body(*args):
        operands = list(args)
        if partition_name is not None:
            operands.append(b2j.partition_id_tensor())
        outs = b2j._bass_exec_p.bind(
            *operands,
            out_avals=tuple(out_avals),
            in_names=tuple(all_in_names),
            out_names=tuple(out_names),
            lowering_input_output_aliases=(),
            sim_require_finite=True,
            sim_require_nnan=True,
            nc=nc_,
        )
        return tuple(outs)

    n_cores = len(in_maps)
    devices = jax.devices()[:n_cores]
    mesh = Mesh(_np.asarray(devices), ("core",))
    in_specs = (PartitionSpec("core"),) * (n_params + n_outs)
    out_specs = (PartitionSpec("core"),) * len(out_names)
    sharded = jax.jit(
        shard_map(_body, mesh=mesh, in_specs=in_specs, out_specs=out_specs,
                  check_rep=False),
        keep_unused=True,
    )
    per_core = [[_np.asarray(m[name]) for name in in_names] for m in in_maps]
    concat_in = [
        _np.concatenate([per_core[c][i] for c in range(n_cores)], axis=0)
        for i in range(n_params)
    ]
    concat_zeros = [
        _np.zeros((n_cores * z.shape[0], *z.shape[1:]), z.dtype) for z in zero_outs
    ]
    return sharded, concat_in, concat_zeros


def _build_sharded_chain(nc, in_maps, chain):
    """Like _build_sharded but executes the NEFF `chain` times per dispatch,
    serialised by threading the output buffers through as the donated
    zero-output operands."""
    import jax
    import numpy as _np
    from jax.sharding import Mesh, PartitionSpec
    from jax.experimental.shard_map import shard_map
    from concourse import bass2jax as b2j

    b2j.install_neuronx_cc_hook()
    nc_ = nc
    partition_name = nc_.partition_id_tensor.name if nc_.partition_id_tensor else None
    in_names, out_names, out_avals, zero_outs = [], [], [], []
    for alloc in nc_.m.functions[0].allocations:
        if not isinstance(alloc, mybir.MemoryLocationSet):
            continue
        name = alloc.memorylocations[0].name
        if alloc.kind == "ExternalInput":
            if name != partition_name:
                in_names.append(name)
        elif alloc.kind == "ExternalOutput":
            out_names.append(name)
            shape = tuple(alloc.tensor_shape)
            dtype = mybir.dt.np(alloc.dtype)
            out_avals.append(jax.core.ShapedArray(shape, dtype))
            zero_outs.append(_np.zeros(shape, dtype))
    n_params = len(in_names)
    all_in_names = list(in_names) + list(out_names)
    if partition_name is not None:
        all_in_names.append(partition_name)

    def _body(*args):
        ins = list(args[:n_params])
        outs = list(args[n_params:])
        for _ in range(chain):
            operands = ins + outs
            if partition_name is not None:
                operands.append(b2j.partition_id_tensor())
            outs = list(b2j._bass_exec_p.bind(
                *operands,
                out_avals=tuple(out_avals),
                in_names=tuple(all_in_names),
                out_names=tuple(out_names),
                lowering_input_output_aliases=(),
                sim_require_finite=True,
                sim_require_nnan=True,
                nc=nc_,
            ))
        return tuple(outs)

    n_cores = len(in_maps)
    devices = jax.devices()[:n_cores]
    mesh = Mesh(_np.asarray(devices), ("core",))
    in_specs = (PartitionSpec("core"),) * (n_params + len(out_names))
    out_specs = (PartitionSpec("core"),) * len(out_names)
    sharded = jax.jit(
        shard_map(_body, mesh=mesh, in_specs=in_specs, out_specs=out_specs,
                  check_rep=False),
        keep_unused=True,
    )
    per_core = [[_np.asarray(m[name]) for name in in_names] for m in in_maps]
    concat_in = [
        _np.concatenate([per_core[c][i] for c in range(n_cores)], axis=0)
        for i in range(n_params)
    ]
    concat_zeros = [
        _np.zeros((n_cores * z.shape[0], *z.shape[1:]), z.dtype) for z in zero_outs
    ]
    return sharded, concat_in, concat_zeros


def bench_chain(np_inputs, reps=6, chain_hi=5):
    """Marginal per-iteration device time via an in-NEFF repeat loop."""
    import jax, time
    from jax.sharding import Mesh, PartitionSpec, NamedSharding
    c_dt = _dt_matrix()
    in_maps = []
    for r in range(8):
        in_maps.append({
            "sig1": np.ascontiguousarray(np_inputs["reference_signal"][r].reshape(P, NCOL)),
            "sig2": np.ascontiguousarray(np_inputs["other_signal"][r].reshape(P, NCOL)),
            "c_dt": c_dt,
        })
    times = {}
    for chain in (1, chain_hi):
        nc = _get_nc(chain, **BEST_KW)
        fn, ci, cz = _build_sharded(nc, in_maps)
        mesh = Mesh(np.asarray(jax.devices()[:8]), ("core",))
        sh = NamedSharding(mesh, PartitionSpec("core"))
        dev_in = [jax.device_put(a, sh) for a in ci]
        dev_zero = [jax.device_put(a, sh) for a in cz]
        jax.block_until_ready(fn(*dev_in, *dev_zero))
        best = float("inf")
        for _ in range(reps):
            t0 = time.perf_counter()
            jax.block_until_ready(fn(*dev_in, *dev_zero))
            t1 = time.perf_counter()
            best = min(best, t1 - t0)
        times[chain] = best
    marg = (times[chain_hi] - times[1]) / (chain_hi - 1)
    return marg * 1e9, times


def bench(np_inputs, iters=30):
    import jax, time
    nc = _get_nc(1, **BEST_KW)
    c_dt = _dt_matrix()
    in_maps = []
    for r in range(8):
        in_maps.append({
            "sig1": np.ascontiguousarray(np_inputs["reference_signal"][r].reshape(P, NCOL)),
            "sig2": np.ascontiguousarray(np_inputs["other_signal"][r].reshape(P, NCOL)),
            "c_dt": c_dt,
        })
    fn, concat_in, concat_zeros = _build_sharded(nc, in_maps)
    from jax.sharding import Mesh, PartitionSpec, NamedSharding
    mesh = Mesh(np.asarray(jax.devices()[:8]), ("core",))
    sh = NamedSharding(mesh, PartitionSpec("core"))
    dev_in = [jax.device_put(a, sh) for a in concat_in]
    dev_zero = [jax.device_put(a, sh) for a in concat_zeros]
    jax.block_until_ready(fn(*dev_in, *dev_zero))  # warm/compile
    jax.block_until_ready(fn(*dev_in, *dev_zero))
    t0 = time.perf_counter()
    for _ in range(iters):
        out = fn(*dev_in, *dev_zero)
    jax.block_until_ready(out)
    t1 = time.perf_counter()
    return (t1 - t0) / iters * 1e9


if __name__ == "__main__":
    rng = np.random.default_rng(0)
    a = rng.random((8, 131072), np.float32)
    b = rng.random((8, 131072), np.float32)
    print(kernel(a, b))


def bench_marginal(np_inputs, ra=6, rb=16, rounds=8, iters=50):
    """Per-execution device time: slope of wall time vs in-NEFF repeat count,
    measured on a single core (identical per-core work), best-of interleaved
    rounds to cancel drift."""
    import jax, time
    from concourse import bass2jax as b2j
    c_dt = _dt_matrix()
    in_map = {"sig1": np.ascontiguousarray(np_inputs["reference_signal"][0].reshape(P, NCOL)),
              "sig2": np.ascontiguousarray(np_inputs["other_signal"][0].reshape(P, NCOL)),
              "c_dt": c_dt}

    def build_one(nc):
        b2j.install_neuronx_cc_hook()
        partition_name = nc.partition_id_tensor.name if nc.partition_id_tensor else None
        in_names, out_names, out_avals, zero_outs = [], [], [], []
        for alloc in nc.m.functions[0].allocations:
            if not isinstance(alloc, mybir.MemoryLocationSet):
                continue
            name = alloc.memorylocations[0].name
            if alloc.kind == "ExternalInput":
                if name != partition_name:
                    in_names.append(name)
            elif alloc.kind == "ExternalOutput":
                out_names.append(name)
                shape = tuple(alloc.tensor_shape)
                dtype = mybir.dt.np(alloc.dtype)
                out_avals.append(jax.core.ShapedArray(shape, dtype))
                zero_outs.append(np.zeros(shape, dtype))
        all_in = list(in_names) + list(out_names)
        if partition_name is not None:
            all_in.append(partition_name)

        def _body(*args):
            operands = list(args)
            if partition_name is not None:
                operands.append(b2j.partition_id_tensor())
            return tuple(b2j._bass_exec_p.bind(
                *operands, out_avals=tuple(out_avals), in_names=tuple(all_in),
                out_names=tuple(out_names), lowering_input_output_aliases=(),
                sim_require_finite=True, sim_require_nnan=True, nc=nc))

        fn = jax.jit(_body, keep_unused=True)
        args = [np.asarray(in_map[n]) for n in in_names] + zero_outs
        dargs = [jax.device_put(a, jax.devices()[0]) for a in args]
        return fn, dargs

    fns = {}
    for rep in (ra, rb):
        fn, dargs = build_one(build_nc(rep, **BEST_KW))
        jax.block_until_ready(fn(*dargs))
        fns[rep] = (fn, dargs)
    best = {rep: float("inf") for rep in fns}
    for _ in range(rounds):
        for rep, (fn, dargs) in fns.items():
            t0 = time.perf_counter()
            for _ in range(iters):
                out = fn(*dargs)
            jax.block_until_ready(out)
            t1 = time.perf_counter()
            best[rep] = min(best[rep], (t1 - t0) / iters)
    return (best[rb] - best[ra]) / (rb - ra) * 1e9



# revision 2
# speedup vs baseline: 1.2885x; 1.2885x over previous
"""Trainium2 Bass kernel v5 for nn_EntropyBasedLossBase.

v4 = v3's proven matmul geometry + post-processing, expansion rebuilt:

- Z2[k] holds z' duplicated pairwise ([z0,z0,z1,z1,...]); reading it with
  an AP whose last dim is [1,2] (packed) makes the chunk-major broadcast
  2x_1p-eligible for TensorTensor (v3's broadcast TT ran at 1x).
- t0d (bins 1..16) built by ONE TT from iota0d and the Z2 broadcast.
- DVE units: 4 staircase TS (add 16*blk, max 0) @4x writing the t tile,
  then one TS (min 1) @4x writing the S tile. ACT units: 4 Relu
  activations with bias=+-16*blk, then Relu(1-X) - both passes entirely
  on the scalar engine, landing direct S values (no sign fixups).
- comb (normalize) on ACT via scale/bias APs; |u| on DVE via abs_max;
  min/max norm via DVE X-reduce + gpsimd partition_all_reduce.
"""
import sys

sys.path.insert(0, "/opt/trn_rl_repo")

from contextlib import ExitStack

import numpy as np

import concourse.bacc as bacc
import concourse.bass as bass
import concourse.bass_isa as bass_isa
import concourse.tile as tile
from concourse import mybir
from concourse.bass_utils import run_bass_kernel_spmd

F32 = mybir.dt.float32
F16 = mybir.dt.float16
OP = mybir.AluOpType
ACT = mybir.ActivationFunctionType

NB = 64            # num bins
P = 128            # partitions
NCOL = 1024        # free dim of the compact [128, 1024] layout (N = P*NCOL)
EPS = float(np.finfo(np.float32).eps)
MAGIC16 = 1536.0   # 1.5 * 2^10: float16 round-to-int shift constant


def _spread_seq(cnt, total=32):
    if cnt <= 0:
        return set()
    step = total / cnt
    s = set()
    x = 0.0
    while len(s) < cnt:
        s.add(min(total - 1, int(x)))
        x += step
    return s


def build_nc(repeat=1, gch=64, npsum=6, act_clamps=10, hyb=0, eb=3, sb=4, b0=16,
             z2_act=1, comb_act=True, do_mm=True, dve_reduce=True):
    GCH = gch
    NGROUP = NCOL // GCH
    NPAIR = GCH // 2
    NU = 2 * NGROUP
    W = NB * GCH                  # dense t-tile width (chunk-major: c*NB + r)
    RW = 2 * NB + 2               # rhs width per pair (128 S cols + 2 ones)
    NBLK = NB // b0
    nc = bacc.Bacc("TRN2", num_devices=8)

    sig1 = nc.dram_tensor("sig1", [P, NCOL], F32, kind="ExternalInput")
    sig2 = nc.dram_tensor("sig2", [P, NCOL], F32, kind="ExternalInput")
    c_dt = nc.dram_tensor("c_dt", [NB, NB], F32, kind="ExternalInput")
    out_h = nc.dram_tensor("out_h", [1, 4], F32, kind="ExternalOutput")

    act_set = _spread_seq(act_clamps, NU)
    hyb_set = set()
    if hyb > 0:
        rest = [u for u in range(NU) if u not in act_set]
        step = max(1, len(rest) // hyb)
        hyb_set = set(rest[::step][:hyb])

    with ExitStack() as ctx:
        tc = ctx.enter_context(tile.TileContext(nc))
        singles = ctx.enter_context(tc.tile_pool(name="singles", bufs=1))
        comp = ctx.enter_context(tc.tile_pool(name="comp", bufs=1))
        iopool = ctx.enter_context(tc.tile_pool(name="iopool", bufs=2))
        texp = ctx.enter_context(tc.tile_pool(name="texp", bufs=eb))
        sexp = ctx.enter_context(tc.tile_pool(name="sexp", bufs=eb))
        psum = ctx.enter_context(tc.tile_pool(name="psum", bufs=1, space="PSUM"))
        post = ctx.enter_context(tc.tile_pool(name="post", bufs=1))
        postp = ctx.enter_context(tc.tile_pool(name="postp", bufs=1, space="PSUM"))

        # ---- constants ----
        # chunk-major b0-iota: value (r + 1) at offset c*b0 + r  (form A)
        iota0d = singles.tile([P, b0 * GCH], F16)
        nc.gpsimd.iota(iota0d[:], pattern=[[0, GCH], [1, b0]], base=1,
                       channel_multiplier=0, allow_small_or_imprecise_dtypes=True)
        # value (r - 1 + 1) = r at offset c*b0 + r: for form B (z+1-iota)
        iota0dm = singles.tile([P, b0 * GCH], F16)
        nc.gpsimd.iota(iota0dm[:], pattern=[[0, GCH], [1, b0]], base=0,
                       channel_multiplier=0, allow_small_or_imprecise_dtypes=True)
        s2bufs = []
        for sb_i in range(sb):
            s2b = singles.tile([P, NPAIR * RW], F16, name=f"s2buf{sb_i}")
            ones_ap = bass.AP(s2b.tensor, s2b.offset + 2 * NB,
                              [s2b.ap[0], [RW, NPAIR], [1, 2]])
            nc.vector.memset(ones_ap, 1.0)
            s2bufs.append(s2b)
        dtm = singles.tile([NB, NB], F32)
        ones_col = singles.tile([NB, 1], F32)
        nc.vector.memset(ones_col[:], 1.0)
        # [P,1] f32 bias constants for the ACT staircase (bias must be an AP)
        bias_aps = {}
        for blk in range(1, NBLK):
            for sgn in (1.0, -1.0):
                val = sgn * b0 * blk
                bt = singles.tile([P, 1], F32, name=f"bias_{val:+.0f}")
                nc.vector.memset(bt[:], float(val))
                bias_aps[val] = bt[:]
        # ACT table warm tile (warms are emitted after the v loads so the
        # scalar-queue DMA trigger for sig2 isn't stuck behind table loads)
        warm = singles.tile([1, 1], F32, name="actwarm")
        nc.vector.memset(warm[:], 1.0)
        # eye_top[p, i] = (p == i), eye_bot[p, i] = (p == i + 64): fold
        # matrices for the PE row-fold of acc (replaces an SBUF-SBUF DMA)
        eye_top = singles.tile([P, NB], F32, name="eye_top")
        eye_bot = singles.tile([P, NB], F32, name="eye_bot")
        eyei = singles.tile([P, NB], F32, name="eyei")
        nc.gpsimd.iota(eyei[:], pattern=[[1, NB]], base=0, channel_multiplier=-1,
                       allow_small_or_imprecise_dtypes=True)
        nc.vector.tensor_scalar(out=eye_top[:], in0=eyei[:], scalar1=0.0,
                                scalar2=None, op0=OP.is_equal)
        nc.vector.tensor_scalar(out=eye_bot[:], in0=eyei[:], scalar1=-64.0,
                                scalar2=None, op0=OP.is_equal)

        def z2_bc(z2t, g):
            """pair-duplicated z read as chunk-major b0-broadcast:
            [[2, GCH], [0, b0//2], [1, 2]] - packed last dim (2x_1p for TT)."""
            off = z2t.offset + g * 2 * GCH
            return bass.AP(z2t.tensor, off,
                           [z2t.ap[0], [2, GCH], [0, b0 // 2], [1, 2]])

        def slab(t, r0, nb_):
            """bins [r0, r0+nb_) of all chunks in a chunk-major [P, W] tile."""
            return bass.AP(t.tensor, t.offset + r0, [t.ap[0], [NB, GCH], [1, nb_]])

        for _rep in range(repeat):
            # ---- load + per-sample compact pass (both signals) ----
            comb = iopool.tile([P, 2 * NCOL], F16, tag="comb")   # s for both signals
            vs = []
            for k, sig in enumerate((sig1, sig2)):
                v = iopool.tile([P, NCOL], F32, tag=f"v{k}")
                # separate queues so the two loads overlap
                eng = nc.sync if k == 0 else nc.scalar
                eng.dma_start(out=v[:], in_=sig.ap())
                vs.append(v)
            if _rep == 0:
                # preload ACT function tables (~1.3us each) under the DMA head
                for fn in (ACT.Identity, ACT.Relu, ACT.Abs):
                    nc.scalar.activation(out=warm[:], in_=warm[:], func=fn)
            z2s = []
            for k in range(2):
                v = vs[k]

                mxa = comp.tile([P, 1], F32, tag=f"mxa{k}")
                mna = comp.tile([P, 1], F32, tag=f"mna{k}")
                if dve_reduce == "pool":
                    # XYZWC reduce only supports add/average/max on HW:
                    # max on Pool; min via ACT negate + Pool max (DVE stays free)
                    mx1 = comp.tile([1, 1], F32, tag=f"mx1{k}")
                    nc.gpsimd.tensor_reduce(out=mx1[:], in_=v[:],
                                            axis=mybir.AxisListType.XYZWC, op=OP.max)
                    nc.gpsimd.partition_broadcast(mxa[:], mx1[:])
                    nv = comp.tile([P, NCOL], F32, tag=f"nv{k}")
                    nc.scalar.activation(out=nv[:], in_=v[:], func=ACT.Copy, scale=-1.0)
                    mn1 = comp.tile([1, 1], F32, tag=f"mn1{k}")
                    nc.gpsimd.tensor_reduce(out=mn1[:], in_=nv[:],
                                            axis=mybir.AxisListType.XYZWC, op=OP.max)
                    mnn = comp.tile([P, 1], F32, tag=f"mnn{k}")
                    nc.gpsimd.partition_broadcast(mnn[:], mn1[:])
                    nc.vector.tensor_scalar(out=mna[:], in0=mnn[:], scalar1=-1.0,
                                            scalar2=None, op0=OP.mult)
                elif dve_reduce:
                    mxp = comp.tile([P, 1], F32, tag=f"mxp{k}")
                    mnp = comp.tile([P, 1], F32, tag=f"mnp{k}")
                    nc.vector.tensor_reduce(out=mxp[:], in_=v[:],
                                            axis=mybir.AxisListType.X, op=OP.max)
                    nc.vector.tensor_reduce(out=mnp[:], in_=v[:],
                                            axis=mybir.AxisListType.X, op=OP.min)
                    nc.vector.tensor_scalar(out=mnp[:], in0=mnp[:], scalar1=-1.0,
                                            scalar2=None, op0=OP.mult)
                    nc.gpsimd.partition_all_reduce(mxa[:], mxp[:], channels=P,
                                                   reduce_op=bass_isa.ReduceOp.max)
                    mnn = comp.tile([P, 1], F32, tag=f"mnn{k}")
                    nc.gpsimd.partition_all_reduce(mnn[:], mnp[:], channels=P,
                                                   reduce_op=bass_isa.ReduceOp.max)
                    nc.vector.tensor_scalar(out=mna[:], in0=mnn[:], scalar1=-1.0,
                                            scalar2=None, op0=OP.mult)
                else:
                    mx1 = comp.tile([1, 1], F32, tag=f"mx1{k}")
                    mn1 = comp.tile([1, 1], F32, tag=f"mn1{k}")
                    nc.gpsimd.tensor_reduce(out=mx1[:], in_=v[:],
                                            axis=mybir.AxisListType.XYZWC, op=OP.max)
                    nv = comp.tile([P, NCOL], F32, tag=f"nv{k}")
                    nc.scalar.activation(out=nv[:], in_=v[:], func=ACT.Copy, scale=-1.0)
                    nc.gpsimd.tensor_reduce(out=mn1[:], in_=nv[:],
                                            axis=mybir.AxisListType.XYZWC, op=OP.max)
                    mnn = comp.tile([P, 1], F32, tag=f"mnn{k}")
                    nc.gpsimd.partition_broadcast(mxa[:], mx1[:])
                    nc.gpsimd.partition_broadcast(mnn[:], mn1[:])
                    nc.vector.tensor_scalar(out=mna[:], in0=mnn[:], scalar1=-1.0,
                                            scalar2=None, op0=OP.mult)

                diff = comp.tile([P, 1], F32, tag=f"diff{k}")
                nc.vector.tensor_tensor(out=diff[:], in0=mxa[:], in1=mna[:], op=OP.subtract)
                rdiff = comp.tile([P, 1], F32, tag=f"rdiff{k}")
                nc.vector.reciprocal(out=rdiff[:], in_=diff[:])
                guard = comp.tile([P, 1], F32, tag=f"guard{k}")
                nc.vector.tensor_scalar(out=guard[:], in0=diff[:], scalar1=EPS,
                                        scalar2=None, op0=OP.is_gt)
                rs = comp.tile([P, 1], F32, tag=f"rs{k}")
                nc.vector.tensor_scalar(out=rs[:], in0=rdiff[:], scalar1=float(NB),
                                        scalar2=guard[:], op0=OP.mult, op1=OP.mult)

                # s = (v - mn) * rscale in [0, 64]  (f16 out)
                cslice = comb[:, k * NCOL:(k + 1) * NCOL]
                if comb_act:
                    nbias = comp.tile([P, 1], F32, tag=f"nbias{k}")
                    nc.vector.tensor_scalar(out=nbias[:], in0=mna[:], scalar1=rs[:],
                                            scalar2=-1.0, op0=OP.mult, op1=OP.mult)
                    nc.scalar.activation(out=cslice, in_=v[:], func=ACT.Identity,
                                         scale=rs[:], bias=nbias[:])
                else:
                    nc.vector.tensor_scalar(out=cslice, in0=v[:], scalar1=mna[:],
                                            scalar2=rs[:], op0=OP.subtract, op1=OP.mult)

                # z' = s + 0.9u - 1.8u|u|, u = frac(s) - 0.5; f16 chain per
                # signal, split in column halves so the first half's z2 (and
                # the first groups' expansion) starts earlier
                z2 = iopool.tile([P, 2 * NCOL], F16, tag=f"z2_{k}")
                HC = NCOL // 2
                for h in range(2):
                    cs = comb[:, k * NCOL + h * HC: k * NCOL + (h + 1) * HC]
                    b1 = comp.tile([P, HC], F16, tag=f"B{k}{h}")
                    nc.vector.tensor_scalar(out=b1[:], in0=cs, scalar1=MAGIC16 - 0.5,
                                            scalar2=None, op0=OP.add)
                    bb = comp.tile([P, HC], F16, tag=f"C{k}{h}")
                    nc.vector.tensor_scalar(out=bb[:], in0=b1[:], scalar1=-MAGIC16 + 0.5,
                                            scalar2=None, op0=OP.add)   # rhe(s-.5)+.5
                    nu = comp.tile([P, HC], F16, tag=f"D{k}{h}")
                    nc.vector.tensor_tensor(out=nu[:], in0=bb[:], in1=cs, op=OP.subtract)
                    au = comp.tile([P, HC], F16, tag=f"B{k}{h}")
                    nc.scalar.activation(out=au[:], in_=nu[:], func=ACT.Abs)  # = |u|
                    q = comp.tile([P, HC], F16, tag=f"C{k}{h}")
                    nc.vector.tensor_scalar(out=q[:], in0=au[:], scalar1=-1.8, scalar2=0.9,
                                            op0=OP.mult, op1=OP.add)    # = 0.9 - 1.8|u|
                    m1 = comp.tile([P, HC], F16, tag=f"B{k}{h}")
                    nc.vector.tensor_tensor(out=m1[:], in0=nu[:], in1=q[:], op=OP.mult)
                    zc16 = comp.tile([P, HC], F16, tag=f"zc16_{k}{h}")
                    nc.vector.tensor_tensor(out=zc16[:], in0=cs, in1=m1[:], op=OP.subtract)
                    # pair-duplicated z half
                    zsrc = bass.AP(zc16.tensor, zc16.offset,
                                   [zc16.ap[0], [1, HC], [0, 2]])
                    zdst = bass.AP(z2.tensor, z2.offset + h * NCOL,
                                   [z2.ap[0], [2, HC], [1, 2]])
                    if k < z2_act:
                        nc.scalar.activation(out=zdst, in_=zsrc, func=ACT.Copy)
                    else:
                        nc.vector.tensor_copy(out=zdst, in_=zsrc)
                z2s.append(z2)

            if not do_mm:
                hout = post.tile([1, 4], F32, tag="hout_ab")
                nc.vector.memset(hout[:], 0.0)
                nc.sync.dma_start(out=out_h.ap(), in_=hout[:])
                continue

            # ---- expansion + matmul over groups ----
            mps = []
            for j in range(npsum):
                mtile = psum.tile([P, RW], F32, tag=f"mps{j}", name=f"mps{j}")
                mps.append(mtile)
            n_mm = NGROUP * NPAIR
            mm_idx = 0
            for g in range(NGROUP):
                if g == 2 and _rep == 0:
                    # Ln table load off the head critical path, well before post
                    nc.scalar.activation(out=warm[:], in_=warm[:], func=ACT.Ln)
                outs = {}
                korder = sorted(range(2), key=lambda k: (2 * g + k) not in act_set)
                for k in korder:
                    u = 2 * g + k
                    on_act = u in act_set
                    if k == 0:
                        st = sexp.tile([P, W], F16, tag="s1t")
                        out_ap = st[:]
                    else:
                        st = s2bufs[g % sb]
                        out_ap = bass.AP(st.tensor, st.offset,
                                         [st.ap[0], [RW, NPAIR], [1, 2 * NB]])
                    on_hyb = u in hyb_set
                    tt = texp.tile([P, W], F16, tag=f"t{k}")
                    t0d = texp.tile([P, b0 * GCH], F16, tag=f"t0d{k}")
                    zbc = z2_bc(z2s[k], g)
                    # form A (k=0 DVE, k=1 ACT/hyb): t0d = iota - z, stair +16blk
                    # form B (k=1 DVE, k=0 ACT/hyb): t0d = (z+1) - iota, stair -16blk
                    form_a = (k == 0) != (on_act or on_hyb)
                    if form_a:
                        nc.vector.tensor_tensor(out=t0d[:], in0=iota0d[:], in1=zbc,
                                                op=OP.subtract)
                    else:
                        nc.vector.tensor_tensor(out=t0d[:], in0=zbc, in1=iota0dm[:],
                                                op=OP.subtract)
                    dsign = 1.0 if form_a else -1.0
                    if on_act:
                        # staircase on ACT: X_blk = relu(t0d + dsign*16blk)
                        for blk in range(NBLK):
                            bv = dsign * float(b0 * blk)
                            bias = bias_aps[bv] if bv != 0.0 else 0.0
                            nc.scalar.activation(out=slab(tt, blk * b0, b0),
                                                 in_=t0d[:], func=ACT.Relu,
                                                 bias=bias)
                    else:
                        # staircase on DVE: X_blk = max(t0d + dsign*16blk, 0)
                        for blk in range(NBLK):
                            nc.vector.tensor_scalar(out=slab(tt, blk * b0, b0),
                                                    in0=t0d[:],
                                                    scalar1=dsign * float(b0 * blk),
                                                    scalar2=0.0,
                                                    op0=OP.add, op1=OP.max)
                    if on_act or on_hyb:
                        # S = relu(1 - X): direct clamp result for this form
                        nc.scalar.activation(out=out_ap, in_=tt[:], func=ACT.Relu,
                                             bias=1.0, scale=-1.0)
                    else:
                        # S = min(X, 1)
                        nc.vector.tensor_scalar(out=out_ap, in0=tt[:], scalar1=1.0,
                                                scalar2=None, op0=OP.min)
                    outs[k] = st
                s1t, s2t = outs[0], outs[1]
                for m in range(NPAIR):
                    j = mm_idx % npsum
                    nc.tensor.matmul(
                        out=mps[j][:],
                        lhsT=s1t[:, m * 2 * NB:(m + 1) * 2 * NB],
                        rhs=s2t[:, m * RW:(m + 1) * RW],
                        start=(mm_idx < npsum), stop=(mm_idx >= n_mm - npsum),
                    )
                    mm_idx += 1

            if _rep == 0:
                # dtm load deferred off the head SP queue (needed only here)
                nc.sync.dma_start(out=dtm[:], in_=c_dt.ap())
            # ---- combine psum tiles ----
            acc = post.tile([P, RW], F32)
            nc.vector.tensor_copy(out=acc[:], in_=mps[0][:])
            for j in range(1, len(mps)):
                nc.vector.tensor_tensor(out=acc[:], in0=mps[j][:], in1=acc[:], op=OP.add)
            # PE fold: jf[i, 0:NB] = Mt = acc[i, 0:NB] + acc[i+64, NB:2NB],
            #          jf[i, NB]   = R1 = acc[i, 2NB] + acc[i+64, 2NB]
            jf = postp.tile([NB, NB + 1], F32, name="jf")
            nc.tensor.matmul(out=jf[:, 0:NB], lhsT=eye_top[:], rhs=acc[:, 0:NB],
                             start=True, stop=False)
            nc.tensor.matmul(out=jf[:, 0:NB], lhsT=eye_bot[:], rhs=acc[:, NB:2 * NB],
                             start=False, stop=True)
            nc.tensor.matmul(out=jf[:, NB:NB + 1], lhsT=eye_top[:],
                             rhs=acc[:, 2 * NB:2 * NB + 1], start=True, stop=False)
            nc.tensor.matmul(out=jf[:, NB:NB + 1], lhsT=eye_bot[:],
                             rhs=acc[:, 2 * NB:2 * NB + 1], start=False, stop=True)
            # jcr = [coldiff(Mt) | R1]  (copy PSUM->SBUF first: TT can read
            # at most one PSUM operand)
            jfs = post.tile([NB, NB + 1], F32, tag="jfs")
            nc.vector.tensor_copy(out=jfs[:], in_=jf[:])
            jcr = post.tile([NB, NB + 1], F32)
            nc.vector.tensor_copy(out=jcr[:, 0:1], in_=jfs[:, 0:1])
            nc.vector.tensor_tensor(out=jcr[:, 1:NB], in0=jfs[:, 1:NB], in1=jfs[:, 0:NB - 1],
                                    op=OP.subtract)
            nc.vector.tensor_copy(out=jcr[:, NB:NB + 1], in_=jfs[:, NB:NB + 1])
            # [D coldiff(Mt) | D R1]
            jps = postp.tile([NB, NB + 1], F32)
            nc.tensor.matmul(out=jps[:], lhsT=dtm[:], rhs=jcr[:], start=True, stop=True)
            jsb = post.tile([NB, NB], F32)
            # joint = (D R1) e0^T - D coldiff(Mt) D^T
            nc.vector.tensor_scalar(out=jsb[:], in0=jps[:, 0:NB], scalar1=-1.0, scalar2=None,
                                    op0=OP.mult)
            nc.vector.tensor_tensor(out=jsb[:, 0:1], in0=jps[:, NB:NB + 1], in1=jsb[:, 0:1],
                                    op=OP.add)

            # ---- clip, sums, entropies ----
            cj = post.tile([NB, NB], F32)
            rowsum = post.tile([NB, 1], F32)
            nc.vector.tensor_scalar(out=cj[:], in0=jsb[:], scalar1=EPS, scalar2=None,
                                    op0=OP.max, op1=OP.add, accum_out=rowsum[:])
            tot = post.tile([NB, 1], F32)
            nc.gpsimd.partition_all_reduce(tot[:], rowsum[:], channels=NB,
                                           reduce_op=bass_isa.ReduceOp.add)

            ly = post.tile([NB, 1], F32)
            nc.scalar.activation(out=ly[:], in_=rowsum[:], func=ACT.Ln)
            cly = post.tile([NB, 1], F32)
            nc.vector.tensor_tensor(out=cly[:], in0=rowsum[:], in1=ly[:], op=OP.mult)
            sy = post.tile([NB, 1], F32)
            nc.gpsimd.partition_all_reduce(sy[:], cly[:], channels=NB,
                                           reduce_op=bass_isa.ReduceOp.add)

            lj = post.tile([NB, NB], F32)
            nc.scalar.activation(out=lj[:], in_=cj[:], func=ACT.Ln)
            clj = post.tile([NB, NB], F32)
            rowsum_cl = post.tile([NB, 1], F32)
            nc.vector.tensor_tensor(out=clj[:], in0=cj[:], in1=lj[:], op=OP.mult)
            nc.vector.tensor_reduce(out=rowsum_cl[:], in_=clj[:], axis=mybir.AxisListType.X, op=OP.add)
            sxy = post.tile([NB, 1], F32)
            nc.gpsimd.partition_all_reduce(sxy[:], rowsum_cl[:], channels=NB,
                                           reduce_op=bass_isa.ReduceOp.add)

            pxp = postp.tile([1, NB], F32)
            nc.tensor.matmul(out=pxp[:], lhsT=ones_col[:], rhs=cj[:], start=True, stop=True)
            px = post.tile([1, NB], F32)
            nc.vector.tensor_copy(out=px[:], in_=pxp[:])
            lx = post.tile([1, NB], F32)
            nc.scalar.activation(out=lx[:], in_=px[:], func=ACT.Ln)
            clx = post.tile([1, NB], F32)
            sx = post.tile([1, 1], F32)
            nc.vector.tensor_tensor(out=clx[:], in0=px[:], in1=lx[:], op=OP.mult)
            nc.vector.tensor_reduce(out=sx[:], in_=clx[:], axis=mybir.AxisListType.X, op=OP.add)

            lnT = post.tile([1, 1], F32)
            nc.scalar.activation(out=lnT[:], in_=tot[0:1, 0:1], func=ACT.Ln)
            rT = post.tile([1, 1], F32)
            nc.vector.reciprocal(out=rT[:], in_=tot[0:1, 0:1])

            hout = post.tile([1, 4], F32)
            for col, sv in ((0, sx[0:1, 0:1]), (1, sy[0:1, 0:1]), (2, sxy[0:1, 0:1])):
                tmp = post.tile([1, 1], F32, tag=f"tmp{col}")
                nc.vector.tensor_tensor(out=tmp[:], in0=sv, in1=rT[:], op=OP.mult)
                nc.vector.tensor_tensor(out=hout[:, col:col + 1], in0=lnT[:], in1=tmp[:],
                                        op=OP.subtract)
            nc.vector.memset(hout[:, 3:4], 0.0)
            nc.sync.dma_start(out=out_h.ap(), in_=hout[:])

    nc.compile()
    return nc


BEST_KW = {"gch": 64, "npsum": 3, "act_clamps": 9, "hyb": 0, "eb": 3, "sb": 4, "b0": 16,
           "z2_act": 2, "comb_act": True, "dve_reduce": "pool"}

_NC_CACHE = {}


def _get_nc(repeat=1, **kw):
    key = (repeat, tuple(sorted(kw.items())))
    if key not in _NC_CACHE:
        _NC_CACHE[key] = build_nc(repeat, **kw)
    return _NC_CACHE[key]


def _dt_matrix():
    # c_dt[k, m] = D[m, k] with D = I - subdiag  (joint = D @ coldiff(M))
    d = np.zeros((NB, NB), np.float32)
    for k in range(NB):
        d[k, k] = 1.0
        if k + 1 < NB:
            d[k, k + 1] = -1.0
    return d


def kernel(reference_signal: np.ndarray, other_signal: np.ndarray):
    B, N = reference_signal.shape
    assert (B, N) == (8, 131072)
    nc = _get_nc(1, **BEST_KW)
    c_dt = _dt_matrix()
    in_maps = []
    for r in range(B):
        in_maps.append({
            "sig1": np.ascontiguousarray(reference_signal[r].reshape(P, NCOL)),
            "sig2": np.ascontiguousarray(other_signal[r].reshape(P, NCOL)),
            "c_dt": c_dt,
        })
    res = run_bass_kernel_spmd(nc, in_maps, list(range(8)))
    hx = np.empty(B, np.float32)
    hy = np.empty(B, np.float32)
    hxy = np.empty(B, np.float32)
    for r in range(B):
        o = res.results[r]["out_h"]
        hx[r], hy[r], hxy[r] = o[0, 0], o[0, 1], o[0, 2]
    return (hx, hy, hxy)


if __name__ == "__main__":
    rng = np.random.default_rng(0)
    a = rng.random((8, 131072), np.float32)
    b = rng.random((8, 131072), np.float32)
    print(kernel(a, b))


def bench_marginal(np_inputs, ra=6, rb=16, rounds=8, iters=50):
    """Per-execution device time: slope of wall time vs in-NEFF repeat count,
    measured on a single core (identical per-core work), best-of interleaved
    rounds to cancel drift."""
    import jax, time
    from concourse import bass2jax as b2j
    from concourse import mybir
    c_dt = _dt_matrix()
    in_map = {"sig1": np.ascontiguousarray(np_inputs["reference_signal"][0].reshape(P, NCOL)),
              "sig2": np.ascontiguousarray(np_inputs["other_signal"][0].reshape(P, NCOL)),
              "c_dt": c_dt}

    def build_one(nc):
        b2j.install_neuronx_cc_hook()
        partition_name = nc.partition_id_tensor.name if nc.partition_id_tensor else None
        in_names, out_names, out_avals, zero_outs = [], [], [], []
        for alloc in nc.m.functions[0].allocations:
            if not isinstance(alloc, mybir.MemoryLocationSet):
                continue
            name = alloc.memorylocations[0].name
            if alloc.kind == "ExternalInput":
                if name != partition_name:
                    in_names.append(name)
            elif alloc.kind == "ExternalOutput":
                out_names.append(name)
                shape = tuple(alloc.tensor_shape)
                dtype = mybir.dt.np(alloc.dtype)
                out_avals.append(jax.core.ShapedArray(shape, dtype))
                zero_outs.append(np.zeros(shape, dtype))
        all_in = list(in_names) + list(out_names)
        if partition_name is not None:
            all_in.append(partition_name)

        def _body(*args):
            operands = list(args)
            if partition_name is not None:
                operands.append(b2j.partition_id_tensor())
            return tuple(b2j._bass_exec_p.bind(
                *operands, out_avals=tuple(out_avals), in_names=tuple(all_in),
                out_names=tuple(out_names), lowering_input_output_aliases=(),
                sim_require_finite=True, sim_require_nnan=True, nc=nc))

        fn = jax.jit(_body, keep_unused=True)
        args = [np.asarray(in_map[n]) for n in in_names] + zero_outs
        dargs = [jax.device_put(a, jax.devices()[0]) for a in args]
        return fn, dargs

    fns = {}
    for rep in (ra, rb):
        fn, dargs = build_one(build_nc(rep, **BEST_KW))
        jax.block_until_ready(fn(*dargs))
        fns[rep] = (fn, dargs)
    best = {rep: float("inf") for rep in fns}
    for _ in range(rounds):
        for rep, (fn, dargs) in fns.items():
            t0 = time.perf_counter()
            for _ in range(iters):
                out = fn(*dargs)
            jax.block_until_ready(out)
            t1 = time.perf_counter()
            best[rep] = min(best[rep], (t1 - t0) / iters)
    return (best[rb] - best[ra]) / (rb - ra) * 1e9
